# revision 21
# baseline (speedup 1.0000x reference)
"""TRN2 Bass kernel for the 3-way factorization-machine MLP (nn_CP_B_53669911331094).

Data-parallel over 8 NeuronCores: each core handles 1024 of the 8192 batch rows.
The 100k-row embedding tables, W1 and the selection matrices are row-sharded
across the 8 cores (~7.6MB upload per core instead of ~59MB replicated) and
reassembled on device with DRAM->DRAM AllGathers; gather indices are remapped
on host to the packed gathered layout.
Pipeline per core (all matmuls fp16 with fp32 PSUM accumulation):
  gather (fp16 tables, indirect DMA) -> PE transpose -> projections (+bias)
  -> projection rows spilled to DRAM, replicated into [128, B] tiles via
     broadcast (stride-0) plain DMAs on the sync/scalar queues (default
     var "gd"; var "ga" uses gpsimd indirect gathers, var "full" uses PE
     selection-matrix matmuls)
  -> DVE tensor-tensor feature products (SBUF f16 operands, 2x mode)
  -> 96-block W1 matmul accumulation -> ReLU -> W2 -> tanh -> W3 -> +b3.
"""
import os
import numpy as np

LATENT = 128
RED = 64
B = 8192
NCORES = 8
BC = B // NCORES          # 1024 batch rows per core
NT = BC // 128            # 8 tiles of 128 rows
NH = BC // 512            # 2 free-dim halves for N=512 matmuls
KB = 3 * RED * RED // 128  # 96 feature k-blocks
TPB = RED * RED // 128     # 32 k-blocks per pair

NUSH = 100000 // NCORES   # user-table rows per core shard
NISH = 100000 // NCORES   # item-table rows per core shard
NTSH = 1000 // NCORES     # time-table rows per core shard
CHUNK = NUSH + NISH + NTSH  # 25125 packed table rows per core

_CACHE = {}


def _build_nc(phases=3, reps=1, var="full", tab8=False, bf16=False):
    import concourse.bass as bass
    import concourse.bacc as bacc
    import concourse.mybir as mybir
    from concourse.tile import TileContext

    f16, f32, i32 = mybir.dt.float16, mybir.dt.float32, mybir.dt.int32
    ftab = mybir.dt.float8e4 if tab8 else f16
    fw = mybir.dt.bfloat16 if bf16 else f16  # work/weight dtype
    Relu = mybir.ActivationFunctionType.Relu
    Tanh = mybir.ActivationFunctionType.Tanh

    nc = bacc.Bacc("TRN2", target_bir_lowering=False, debug=False,
                   num_devices=NCORES, num_swdge_queues=4)

    tabs_d = nc.dram_tensor("tabs", [CHUNK, LATENT], ftab, kind="ExternalInput")
    idx_d = nc.dram_tensor("idx", [3, 128, NT], i32, kind="ExternalInput")
    wproj_d = nc.dram_tensor("wproj", [128, 320], fw, kind="ExternalInput")
    bproj_d = nc.dram_tensor("bproj", [128, 3], f32, kind="ExternalInput")
    w1ts_d = nc.dram_tensor("w1ts", [128 // NCORES, KB * 256], fw,
                            kind="ExternalInput")
    b1t_d = nc.dram_tensor("b1t", [128, 2], f32, kind="ExternalInput")
    w2t_d = nc.dram_tensor("w2t", [128, 256], fw, kind="ExternalInput")
    b2t_d = nc.dram_tensor("b2t", [128, 1], f32, kind="ExternalInput")
    w3t_d = nc.dram_tensor("w3t", [128, 1], fw, kind="ExternalInput")
    b3_d = nc.dram_tensor("b3", [1, 1], f32, kind="ExternalInput")
    eye_d = nc.dram_tensor("eye", [128, 128], f16, kind="ExternalInput")
    sels_d = nc.dram_tensor("sels", [128 // NCORES, TPB * 128], fw,
                            kind="ExternalInput")
    idxr_d = nc.dram_tensor("idxr", [128, 2 * TPB], i32, kind="ExternalInput")
    out_d = nc.dram_tensor("out", [1, BC], f32, kind="ExternalOutput")
    spills = [nc.dram_tensor(f"spill{r}", [128, BC], fw)
              for r in range(reps)]

    RG = [list(range(NCORES))]

    with TileContext(nc) as tc:
        with tc.tile_pool(name="dram", bufs=1, space="DRAM") as dp, \
             tc.tile_pool(name="const", bufs=1) as cp, \
             tc.tile_pool(name="work", bufs=1) as wp:
            # ---- reassemble row-sharded inputs with DRAM->DRAM AllGathers ----
            # (collectives can't touch I/O tensors, so bounce via internal DRAM)
            tabs_b = dp.tile([CHUNK, LATENT], ftab, tag="tabs_b")
            tab_full = dp.tile([NCORES * CHUNK, LATENT], ftab, tag="tab_full",
                               addr_space="Shared")
            w1ts_b = dp.tile([128 // NCORES, KB * 256], fw, tag="w1ts_b")
            w1t_full = dp.tile([128, KB * 256], fw, tag="w1t_full",
                               addr_space="Shared")
            sels_b = dp.tile([128 // NCORES, TPB * 128], fw, tag="sels_b")
            sel_full = dp.tile([128, TPB * 128], fw, tag="sel_full",
                               addr_space="Shared")
            nc.sync.dma_start(tabs_b[:], tabs_d[:])
            nc.sync.dma_start(w1ts_b[:], w1ts_d[:])
            nc.sync.dma_start(sels_b[:], sels_d[:])
            nc.gpsimd.collective_compute(
                "AllGather", mybir.AluOpType.bypass, replica_groups=RG,
                ins=[w1ts_b.opt()], outs=[w1t_full.opt()])
            nc.gpsimd.collective_compute(
                "AllGather", mybir.AluOpType.bypass, replica_groups=RG,
                ins=[sels_b.opt()], outs=[sel_full.opt()])
            nc.gpsimd.collective_compute(
                "AllGather", mybir.AluOpType.bypass, replica_groups=RG,
                ins=[tabs_b.opt()], outs=[tab_full.opt()])

            # ---- resident constants ----
            w1t = cp.tile([128, KB * 256], fw, tag="w1t")
            nc.sync.dma_start(w1t[:], w1t_full[:])
            wproj = cp.tile([128, 320], fw, tag="wproj")
            nc.sync.dma_start(wproj[:], wproj_d[:])
            w2t = cp.tile([128, 256], fw, tag="w2t")
            nc.sync.dma_start(w2t[:], w2t_d[:])
            w3t = cp.tile([128, 1], fw, tag="w3t")
            nc.sync.dma_start(w3t[:], w3t_d[:])
            eye = cp.tile([128, 128], f16, tag="eye")
            nc.sync.dma_start(eye[:], eye_d[:])
            sel = cp.tile([128, TPB * 128], fw, tag="sel")
            nc.sync.dma_start(sel[:], sel_full[:])
            idxr = cp.tile([128, 2 * TPB], i32, tag="idxr")
            nc.sync.dma_start(idxr[:], idxr_d[:])
            idx = cp.tile([128, 3 * NT], i32, tag="idx")
            idx_src = bass.AP(idx_d[:].tensor, 0,
                              [[NT, 128], [128 * NT, 3], [1, NT]])
            nc.sync.dma_start(idx[:], idx_src)

            # biases: engine-local copies (1-wait discipline)
            braw = cp.tile([128, 3], f32, tag="braw")
            nc.sync.dma_start(braw[:], bproj_d[:])
            bproj = cp.tile([128, 3], f32, tag="bproj")
            nc.vector.tensor_copy(bproj[:], braw[:])
            b1raw = cp.tile([128, 2], f32, tag="b1raw")
            nc.sync.dma_start(b1raw[:], b1t_d[:])
            b1t = cp.tile([128, 2], f32, tag="b1t")
            nc.scalar.copy(b1t[:], b1raw[:])
            b2raw = cp.tile([128, 1], f32, tag="b2raw")
            nc.sync.dma_start(b2raw[:], b2t_d[:])
            b2t = cp.tile([128, 1], f32, tag="b2t")
            nc.scalar.copy(b2t[:], b2raw[:])
            b3raw = cp.tile([1, 1], f32, tag="b3raw")
            nc.sync.dma_start(b3raw[:], b3_d[:])
            b3 = cp.tile([1, 1], f32, tag="b3")
            nc.vector.tensor_copy(b3[:], b3raw[:])

            # ---- big SBUF work tiles ----
            eT = [wp.tile([128, BC], fw, tag=f"eT{x}", name=f"eT{x}")
                  for x in range(3)]
            iT = wp.tile([128, BC], fw, tag="iT")
            nc.vector.memset(iT[64:128, :], 0.0)
            jj = wp.tile([128, BC], fw, tag="jj")
            kk = wp.tile([128, BC], fw, tag="kk")
            h1 = [wp.tile([128, BC], fw, tag=f"h1{o}", name=f"h1{o}")
                  for o in range(2)]
            h2 = wp.tile([128, BC], fw, tag="h2")
            out_sb = wp.tile([1, BC], f32, tag="out_sb")

            for _rep in range(reps):
              # ---- phase 1: gathers + transposes + projections ----
                with tc.tile_pool(name=f"ps1_{_rep}", bufs=1, space="PSUM") as ps1, \
                   tc.tile_pool(name=f"gp_{_rep}", bufs=8) as gp:
                  pj = []  # projection psums
                  for x in range(3):
                      for t in range(NT):
                          g = gp.tile([128, 128], ftab, tag="g")
                          if var == "nogather":
                              nc.sync.dma_start(g[:], tab_full[0:128, :])
                          else:
                              nc.gpsimd.indirect_dma_start(
                                  out=g[:], out_offset=None, in_=tab_full[:],
                                  in_offset=bass.IndirectOffsetOnAxis(
                                      ap=idx[:, x * NT + t: x * NT + t + 1], axis=0))
                          if tab8:
                              g16 = gp.tile([128, 128], f16, tag="g16")
                              nc.scalar.copy(g16[:], g[:])
                              g = g16
                          tp = ps1.tile([128, 128], f16, tag="tp", bufs=2)
                          nc.tensor.transpose(tp[:], g[:], eye[:])
                          nc.vector.tensor_copy(eT[x][:, t * 128:(t + 1) * 128], tp[:])
                      # projection for table x
                      if x == 0:
                          p = ps1.tile([64, BC], f32, tag=f"proj{x}", name=f"p{x}")
                          lhsT = wproj[:, 0:64]
                      else:
                          p = ps1.tile([128, BC], f32, tag=f"proj{x}", name=f"p{x}")
                          lhsT = wproj[:, 64 + (x - 1) * 128: 64 + x * 128]
                      for nh in range(NH):
                          nc.tensor.matmul(p[:, nh * 512:(nh + 1) * 512], lhsT,
                                           eT[x][:, nh * 512:(nh + 1) * 512],
                                           start=True, stop=True)
                      pj.append(p)
                  # bias add + cast to fp16
                  nc.vector.tensor_scalar_add(iT[0:64, :], pj[0][:], bproj[0:64, 0:1])
                  nc.vector.tensor_scalar_add(jj[:], pj[1][:], bproj[:, 1:2])
                  nc.vector.tensor_scalar_add(kk[:], pj[2][:], bproj[:, 2:3])

                if phases >= 2:
                  if var[:2] in ("ga", "gd"):
                      spill = spills[_rep]
                      nc.sync.dma_start(spill[0:64, :], iT[0:64, :])
                      nc.scalar.dma_start(spill[64:128, :], jj[0:64, :])
                  # ---- phase 2: feature blocks + W1 accumulation ----
                  # rep tiles come from the PE: rep[p,:] = src[2t + p//64, :]
                  # via constant selection matrices (no DMA traffic at all).
                  with tc.tile_pool(name=f"ps2_{_rep}", bufs=1, space="PSUM") as ps2, \
                       tc.tile_pool(name=f"fp_{_rep}", bufs=8) as fp:
                      w1ps = [[ps2.tile([128, 512], f32, tag=f"w1ps{o}{h}",
                                        name=f"w1ps{o}{h}")
                               for h in range(NH)] for o in range(2)]

                      def w1_mms(kb, ft, start, stop):
                          if "nomm" in var and not (start or stop):
                              return
                          for o in range(2):
                              for h in range(NH):
                                  nc.tensor.matmul(
                                      w1ps[o][h][:],
                                      w1t[:, kb * 256 + o * 128:
                                          kb * 256 + (o + 1) * 128],
                                      ft[:, h * 512:(h + 1) * 512],
                                      start=start, stop=stop)

                      with tc.tile_pool(name=f"rps_{_rep}", bufs=4,
                                        space="PSUM") as rps, \
                           tc.tile_pool(name=f"rsb_{_rep}", bufs=4) as rsb:
                        rep_cache = []
                        full_rep = [None]

                        def mk_rep(nm, srcT, t, jsel=0):
                            # rep[p, :] = srcT[2t + p//64, :]
                            if var == "selonce" and rep_cache:
                                return rep_cache[0]
                            if var[:2] in ("ga", "gd"):
                                rg = rsb.tile([128, BC], fw, tag="rs",
                                              name=f"rg{nm}")
                                if var.startswith("gd"):
                                    row = jsel * 64 + 2 * t
                                    eng = [nc.sync, nc.scalar]
                                    for hh in range(2):
                                        bsrc = bass.AP(
                                            spill[:].tensor, (row + hh) * BC,
                                            [[0, 64], [1, BC]])
                                        eng[hh].dma_start(
                                            rg[hh * 64:(hh + 1) * 64, :], bsrc)
                                else:
                                    col = jsel * TPB + t
                                    nc.gpsimd.indirect_dma_start(
                                        out=rg[:], out_offset=None,
                                        in_=spill[:],
                                        in_offset=bass.IndirectOffsetOnAxis(
                                            ap=idxr[:, col:col + 1], axis=0))
                                full_rep[0] = rg
                                return [rg[:, h * 512:(h + 1) * 512]
                                        for h in range(NH)]
                            if var == "gsrep":
                                rg = rsb.tile([128, BC], fw, tag="rs",
                                              name=f"rg{nm}")
                                nc.gpsimd.partition_broadcast(
                                    rg[0:64, :], srcT[2 * t:2 * t + 1, :],
                                    channels=64)
                                nc.gpsimd.partition_broadcast(
                                    rg[64:128, :],
                                    srcT[2 * t + 1:2 * t + 2, :], channels=64)
                                return [rg[:, h * 512:(h + 1) * 512]
                                        for h in range(NH)]
                            rep = [rps.tile([128, 512], f32, tag="rep",
                                            name=f"rp{nm}{h}")
                                   for h in range(NH)]
                            for h in range(NH):
                                if var == "sel128":
                                    nc.tensor.matmul(
                                        rep[h][:],
                                        sel[:, t * 128:(t + 1) * 128],
                                        srcT[:, h * 512:(h + 1) * 512],
                                        start=True, stop=True)
                                else:
                                    nc.tensor.matmul(
                                        rep[h][:],
                                        sel[0:64, t * 128:(t + 1) * 128],
                                        srcT[0:64, h * 512:(h + 1) * 512],
                                        start=True, stop=True)
                            if var == "ftsbuf":
                                rs = [rsb.tile([128, 512], fw, tag="rs",
                                               name=f"rs{nm}{h}")
                                      for h in range(NH)]
                                for h in range(NH):
                                    nc.scalar.copy(rs[h][:], rep[h][:])
                                rep = rs
                            aps = [r[:] for r in rep]
                            if var == "selonce":
                                rep_cache.append(aps)
                            return aps

                        if var in ("full", "sel128"):
                          # unified pipelined schedule: sel(t+1) is emitted
                          # before w1(t) so DVE muls of t+1 overlap w1(t) on
                          # the PE. All matmuls are 128-contraction (sel is
                          # zero-padded) to keep the PE stream uniform.
                          blocks = ([("i", iT, t) for t in range(TPB)] +
                                    [("j", jj, t) for t in range(TPB)])

                          def emit_rep128(b):
                              nm, srcT, t = blocks[b]
                              rp = [rps.tile([128, 512], f32, tag="rep",
                                             name=f"rp{nm}{t}{h}")
                                    for h in range(NH)]
                              for h in range(NH):
                                  nc.tensor.matmul(
                                      rp[h][:],
                                      sel[:, t * 128:(t + 1) * 128],
                                      srcT[:, h * 512:(h + 1) * 512],
                                      start=True, stop=True)
                              return rp

                          pend = {0: emit_rep128(0)}
                          for b in range(2 * TPB):
                              nm, srcT, t = blocks[b]
                              if b + 1 < 2 * TPB:
                                  pend[b + 1] = emit_rep128(b + 1)
                              rp = pend.pop(b)
                              if nm == "i":
                                  ftij = fp.tile([128, BC], fw, tag="ft",
                                                 name=f"ftij{t}")
                                  ftik = fp.tile([128, BC], fw, tag="ft",
                                                 name=f"ftik{t}")
                                  for h in range(NH):
                                      sl = slice(h * 512, (h + 1) * 512)
                                      nc.vector.tensor_mul(
                                          ftij[:, sl], rp[h][:], jj[:, sl])
                                      nc.vector.tensor_mul(
                                          ftik[:, sl], rp[h][:], kk[:, sl])
                                  w1_mms(t, ftij, t == 0, False)
                                  w1_mms(TPB + t, ftik, False, False)
                              else:
                                  ftjk = fp.tile([128, BC], fw, tag="ft",
                                                 name=f"ftjk{t}")
                                  for h in range(NH):
                                      sl = slice(h * 512, (h + 1) * 512)
                                      nc.vector.tensor_mul(
                                          ftjk[:, sl], rp[h][:], kk[:, sl])
                                  w1_mms(2 * TPB + t, ftjk, False,
                                         t == TPB - 1)

                        ft_cache = []
                        for t in (range(TPB) if var not in ("full", "sel128")
                                  else []):
                          if "nott" in var and ft_cache:
                              ftij, ftik = ft_cache[0], ft_cache[1]
                          else:
                              rep = mk_rep(f"i{t}", iT, t)
                              ftij = fp.tile([128, BC], fw, tag="ft")
                              ftik = fp.tile([128, BC], fw, tag="ft")
                              if var[:2] in ("ga", "gd"):
                                  rg = full_rep[0]
                                  nc.vector.tensor_mul(ftij[:], rg[:], jj[:])
                                  nc.vector.tensor_mul(ftik[:], rg[:], kk[:])
                              else:
                                for h in range(NH):
                                  sl = slice(h * 512, (h + 1) * 512)
                                  nc.vector.tensor_mul(ftij[:, sl], rep[h],
                                                       jj[:, sl])
                                  nc.vector.tensor_mul(ftik[:, sl], rep[h],
                                                       kk[:, sl])
                              if "nott" in var:
                                  ft_cache.extend([ftij, ftik])
                          w1_mms(t, ftij, t == 0, False)
                          w1_mms(TPB + t, ftik, False, False)
                        for t in (range(TPB) if var not in ("full", "sel128")
                                  else []):
                          if "nott" in var:
                              ftjk = ft_cache[0]
                          else:
                              rep = mk_rep(f"j{t}", jj, t, jsel=1)
                              ftjk = fp.tile([128, BC], fw, tag="ft")
                              if var[:2] in ("ga", "gd"):
                                  rg = full_rep[0]
                                  nc.vector.tensor_mul(ftjk[:], rg[:], kk[:])
                              else:
                                for h in range(NH):
                                  sl = slice(h * 512, (h + 1) * 512)
                                  nc.vector.tensor_mul(ftjk[:, sl], rep[h],
                                                       kk[:, sl])
                          w1_mms(2 * TPB + t, ftjk, False, t == TPB - 1)

                      # ---- phase 3: MLP head ----
                      ps3cm = tc.tile_pool(name=f"ps3_{_rep}", bufs=1,
                                           space="PSUM")
                      ps3 = ps3cm.__enter__()
                      for o in range(2):
                          for h in range(NH):
                              nc.scalar.activation(
                                  h1[o][:, h * 512:(h + 1) * 512],
                                  w1ps[o][h][:], Relu,
                                  bias=b1t[:, o:o + 1], scale=1.0)
                      for h in range(NH):
                          p2 = ps3.tile([128, 512], f32, tag="w2ps", name="p2")
                          nc.tensor.matmul(p2[:], w2t[:, 0:128],
                                           h1[0][:, h * 512:(h + 1) * 512],
                                           start=True, stop=False)
                          nc.tensor.matmul(p2[:], w2t[:, 128:256],
                                           h1[1][:, h * 512:(h + 1) * 512],
                                           start=False, stop=True)
                          nc.scalar.activation(h2[:, h * 512:(h + 1) * 512], p2[:],
                                               Tanh, bias=b2t[:, 0:1], scale=1.0)
                      for h in range(NH):
                          p3 = ps3.tile([1, 512], f32, tag="w3ps", name="p3")
                          nc.tensor.matmul(p3[:], w3t[:],
                                           h2[:, h * 512:(h + 1) * 512],
                                           start=True, stop=True)
                          nc.vector.tensor_scalar_add(
                              out_sb[:, h * 512:(h + 1) * 512], p3[:], b3[0:1, 0:1])
                      ps3cm.__exit__(None, None, None)
                else:
                  # phase-1 bisect mode: dump a row of jj as the output
                  nc.vector.tensor_copy(out_sb[0:1, :], jj[0:1, :])

            nc.sync.dma_start(out_d[:], out_sb[:])
    nc.compile()
    return nc


def _prep_consts(user_emb, item_emb, time_emb, Wi, bi, Wj, bj, Wk, bk,
                 W1, b1, W2, b2, W3, b3, tab8=False, bf16=False):
    f16 = np.float16
    if bf16:
        import ml_dtypes
        fwk = ml_dtypes.bfloat16
    else:
        fwk = f16
    c = {}
    # packed per-core table shards: [user 12500 | item 12500 | time 125] rows
    # fp8 tables are pre-scaled by 2^7 (into e4m3 normal range); the inverse
    # scale is folded into wproj so the projections come out unscaled.
    if tab8:
        import ml_dtypes
        tdt, tscale, wscale = ml_dtypes.float8_e4m3, 128.0, 1.0 / 128.0
    else:
        tdt, tscale, wscale = f16, 1.0, 1.0
    tab_u = (np.asarray(user_emb, np.float32) * tscale).astype(tdt)
    tab_i = (np.asarray(item_emb, np.float32) * tscale).astype(tdt)
    tab_t = (np.asarray(time_emb, np.float32) * tscale).astype(tdt)
    c["tabs_pc"] = [
        np.concatenate([tab_u[cc * NUSH:(cc + 1) * NUSH],
                        tab_i[cc * NISH:(cc + 1) * NISH],
                        tab_t[cc * NTSH:(cc + 1) * NTSH]], axis=0)
        for cc in range(NCORES)
    ]
    wproj = np.zeros((128, 320), np.float32)
    wproj[:, 0:64] = Wi.T
    wproj[:, 64:128] = Wj.T
    wproj[:, 128:192] = Wj.T
    wproj[:, 192:256] = Wk.T
    wproj[:, 256:320] = Wk.T
    c["wproj"] = (wproj * wscale).astype(fwk)
    bproj = np.zeros((128, 3), np.float32)
    bproj[0:64, 0] = bi
    bproj[:, 1] = np.concatenate([bj, bj])
    bproj[:, 2] = np.concatenate([bk, bk])
    c["bproj"] = bproj
    # W1 [256, 12288] -> lhsT layout [128, 96*256]: block kb = W1.T[kb*128:(kb+1)*128, :]
    # row-sharded: core c uploads partition rows [16c, 16(c+1))
    c["w1t"] = np.ascontiguousarray(
        W1.T.reshape(KB, 128, 256).transpose(1, 0, 2).reshape(128, KB * 256),
        dtype=fwk)
    c["b1t"] = np.ascontiguousarray(b1.reshape(2, 128).T, dtype=np.float32)
    c["w2t"] = np.ascontiguousarray(
        W2.T.reshape(2, 128, 128).transpose(1, 0, 2).reshape(128, 256), dtype=fwk)
    c["b2t"] = np.ascontiguousarray(b2.reshape(128, 1), dtype=np.float32)
    c["w3t"] = np.ascontiguousarray(W3.T, dtype=fwk)
    c["b3"] = np.ascontiguousarray(b3.reshape(1, 1), dtype=np.float32)
    c["eye"] = np.eye(128, dtype=f16)
    # selection matrices: sel[d, t*128 + p] = 1 iff d == 2t + p//64
    sel = np.zeros((128, TPB * 128), fwk)
    for t in range(TPB):
        sel[2 * t, t * 128: t * 128 + 64] = 1
        sel[2 * t + 1, t * 128 + 64: (t + 1) * 128] = 1
    c["sel"] = sel
    # gather-replication indices: col t -> rows {2t}*64,{2t+1}*64 of spill;
    # col 32+t -> same but offset 64 (j rows)
    p = np.arange(128) // 64
    idxr = np.zeros((128, 2 * TPB), np.int32)
    for t in range(TPB):
        idxr[:, t] = 2 * t + p
        idxr[:, TPB + t] = 64 + 2 * t + p
    c["idxr"] = idxr
    return c


def _make_in_maps(consts, i_input, j_input, k_input):
    # remap global table indices to the packed AllGather layout:
    # owner core cc's chunk occupies rows [cc*CHUNK, (cc+1)*CHUNK) with
    # user rows first, then item (+NUSH), then time (+NUSH+NISH).
    ii = np.asarray(i_input).astype(np.int64)
    jjx = np.asarray(j_input).astype(np.int64)
    kkx = np.asarray(k_input).astype(np.int64)
    ii = (ii // NUSH) * CHUNK + (ii % NUSH)
    jjx = (jjx // NISH) * CHUNK + NUSH + (jjx % NISH)
    kkx = (kkx // NTSH) * CHUNK + NUSH + NISH + (kkx % NTSH)
    ii, jjx, kkx = (a.astype(np.int32) for a in (ii, jjx, kkx))
    shared = {k: v for k, v in consts.items()
              if k not in ("tabs_pc", "w1t", "sel")}
    in_maps = []
    nsh = 128 // NCORES
    for c in range(NCORES):
        sl = slice(c * BC, (c + 1) * BC)
        idx = np.stack([
            ii[sl].reshape(NT, 128).T,
            jjx[sl].reshape(NT, 128).T,
            kkx[sl].reshape(NT, 128).T,
        ]).astype(np.int32)  # [3, 128, NT]
        m = dict(shared)
        m["idx"] = np.ascontiguousarray(idx)
        m["tabs"] = consts["tabs_pc"][c]
        m["w1ts"] = np.ascontiguousarray(consts["w1t"][c * nsh:(c + 1) * nsh])
        m["sels"] = np.ascontiguousarray(consts["sel"][c * nsh:(c + 1) * nsh])
        in_maps.append(m)
    return in_maps


def kernel(i_input, j_input, k_input, user_emb, item_emb, time_emb,
           Wi, bi, Wj, bj, Wk, bk, W1, b1, W2, b2, W3, b3):
    from concourse.bass_utils import run_bass_kernel_spmd

    tab8 = os.environ.get("BASS_TAB8", "0") == "1"
    bf16 = os.environ.get("BASS_BF16", "0") == "1"
    consts = _prep_consts(
        np.asarray(user_emb), np.asarray(item_emb), np.asarray(time_emb),
        np.asarray(Wi), np.asarray(bi), np.asarray(Wj), np.asarray(bj),
        np.asarray(Wk), np.asarray(bk), np.asarray(W1), np.asarray(b1),
        np.asarray(W2), np.asarray(b2), np.asarray(W3), np.asarray(b3),
        tab8=tab8, bf16=bf16)

    phases = int(os.environ.get("BASS_PHASES", "3"))
    reps = int(os.environ.get("BASS_REPS", "1"))
    var = os.environ.get("BASS_VAR", "gd")
    key = ("nc", phases, reps, var, tab8, bf16)
    if key not in _CACHE:
        _CACHE[key] = _build_nc(phases, reps, var, tab8=tab8, bf16=bf16)
    nc = _CACHE[key]

    in_maps = _make_in_maps(consts, i_input, j_input, k_input)
    res = run_bass_kernel_spmd(nc, in_maps, list(range(NCORES)))
    out = np.concatenate([res.results[c]["out"][0] for c in range(NCORES)])
    return out.astype(np.float32)

